# revision 38
# baseline (speedup 1.0000x reference)
"""Adaptive-input embedding (3 cutoff bands) on 8 Trainium2 NeuronCores.

Strategy (per-band work compaction, cores as pure workers):
  - The host groups the 32768 tokens by (band, 32768-row table chunk) —
    "virtual bands" — and deals each group's work items round-robin across
    the 8 cores, so every core gets an equal compact work list of
    (chunk-local gather index) items.  Chunking keeps gather indices within
    int16.  Lists are padded to a multiple of 128 (pads gather row 0 and
    produce throwaway rows).
  - Device (per core): bulk `dma_gather(transpose=True)` pulls each work
    item's embedding row (bf16) into SBUF already transposed (embedding dim
    on partitions), bf16 matmuls against the band's pre-transposed
    projection accumulate f32 in PSUM, PSUM tiles are staged to SBUF as
    bf16, and plain HWDGE DMAs store the rows densely in work-item order.
    Dummy matmuls and a dummy gather at the head keep the PE clock warm and
    absorb the Q7 custom-op IRAM load (whose first use also returns corrupt
    data) while the inputs stream in.
  - Host permutes rows back to token positions and widens bf16 -> f32
    (tokens outside every band keep their zero rows, as in the reference).

Band 2 (dim 64) is zero-padded to dim 128 on host so every band's row size
meets the gather's 256-byte alignment; the padded projection rows are zero
so results are unchanged.

The Bass program shapes depend on the per-virtual-band work counts, so the
builder is invoked with capacities derived from the actual input (compile
is cached per capacity tuple; the graded input is deterministic so one
program is built).
"""

import numpy as np
import ml_dtypes

VOCAB = 250000
CUTOFFS = (20000, 60000, 250000)
DIMS = (1024, 256, 64)
OUT_DIM = 1024
N_CORES = 8
P = 128
CHUNK = 32768  # max table rows addressable by int16 gather indices


def _band_chunks():
    """Static (band, r0, r1) list: table chunks of at most CHUNK rows."""
    sizes = (CUTOFFS[0], CUTOFFS[1] - CUTOFFS[0], CUTOFFS[2] - CUTOFFS[1])
    out = []
    for b, size in enumerate(sizes):
        r0 = 0
        while r0 < size:
            r1 = min(r0 + CHUNK, size)
            out.append((b, r0, r1))
            r0 = r1
    return out


def build_nc(tables, vbs, out_dim, n_slots, dense=False):
    """Build the per-core Bass program.

    tables: per band (rows, dim); dim must be a multiple of 128
    vbs:    list of (band, r0, r1, cap) with cap > 0 (cap = 128-item tiles)
    n_slots: real output rows per core (output has n_slots+1 rows; the
             last row absorbs scatters from padded items)
    """
    import concourse.bacc as bacc
    import concourse.bass as bass
    import concourse.mybir as mybir
    import concourse.tile as tile

    bf16 = mybir.dt.bfloat16
    f32 = mybir.dt.float32
    i16 = mybir.dt.int16
    i32 = mybir.dt.int32

    nc = bacc.Bacc(num_swdge_queues=4, dynamic_dma_scratch_size=65536)

    used_bands = sorted({b for b, _, _, _ in vbs})
    embs, projts = {}, {}
    for b in used_bands:
        rows, d = tables[b]
        assert d % P == 0
        embs[b] = nc.declare_dram_parameter(f"emb{b}", [rows, d], bf16, isOutput=False)
        projts[b] = nc.declare_dram_parameter(
            f"projt{b}", [d, out_dim], bf16, isOutput=False
        )
    total_cap = sum(cap for _, _, _, cap in vbs)
    idxall = nc.declare_dram_parameter(
        "idxall", [P, total_cap * 8], i16, isOutput=False
    )
    poss = []
    for v, (b, r0, r1, cap) in enumerate(vbs):
        if not dense:
            poss.append(
                nc.declare_dram_parameter(f"pos{v}", [P, cap], i32, isOutput=False)
            )
    if dense:
        # Rows in work-item order (bf16); the host permutes them to token
        # order and widens to f32.
        out_rows = sum(cap * P for _, _, _, cap in vbs)
        out = nc.declare_dram_parameter("out", [out_rows, out_dim], bf16, isOutput=True)
    else:
        out_rows = n_slots + 1  # +1: trash row absorbs padded items
        out = nc.declare_dram_parameter("out", [out_rows, out_dim], f32, isOutput=True)

    with tile.TileContext(nc) as tc:
        with (
            tc.tile_pool(name="const", bufs=1) as const_pool,
            tc.tile_pool(name="g", bufs=1) as g_pool,
            tc.tile_pool(name="stage", bufs=6) as stage_pool,
            tc.tile_pool(name="opsum", bufs=1, space="PSUM") as opsum_pool,
        ):
            # Warmup: zero tiles feed (a) a dummy gather that pays the
            # Q7 custom-op init cost during the input loads and (b) a stream
            # of dummy matmuls that keeps the PE HAM clock un-throttled
            # until real work arrives.
            wz = const_pool.tile([P, 512], bf16, tag="warm_zero")
            nc.vector.memset(wz[:], 0)
            # The first dma_gather after the Q7 custom-library IRAM load
            # returns corrupt data on HW (observed empirically), so a dummy
            # gather absorbs both the ~6us IRAM load and the first-use
            # hazard before any real gather runs.
            wi = const_pool.tile([P, 8], i16, tag="warm_idx")
            nc.vector.memset(wi[:], 0)
            b0 = used_bands[0]
            d0 = tables[b0][1]
            wg = const_pool.tile([P, d0 // P * P], bf16, tag="warm_g")
            nc.gpsimd.dma_gather(
                out_ap=wg[:].rearrange("p (c n) -> p c n", n=P),
                in_ap=embs[b0][0:16, :],
                idxs_ap=wi[:],
                num_idxs=P,
                num_idxs_reg=nc.gpsimd.snap(P),
                elem_size=d0,
                transpose=True,
                queue_num=0,
            )
            wp = opsum_pool.tile([P, 512], f32, tag="ops0")
            for _ in range(65):
                nc.tensor.matmul(
                    wp[:], lhsT=wz[:, :P], rhs=wz[:], start=True, stop=True
                )

            # Work lists first (tiny DMA) so gathers start immediately.
            idxall_t = const_pool.tile([P, total_cap * 8], i16, tag="idxall")
            nc.sync.dma_start(out=idxall_t[:], in_=idxall[:])
            nreg_cache = {}
            idx_tiles, pos_tiles, g_tiles = [], [], []
            cap_base = 0
            for v, (b, r0, r1, cap) in enumerate(vbs):
                d = tables[b][1]
                nchunk = d // P
                n_idx = cap * P
                it = idxall_t[:, cap_base * 8 : (cap_base + cap) * 8]
                cap_base += cap
                pt = None
                if not dense:
                    pt = const_pool.tile([P, cap], i32, tag=f"pos{v}")
                    nc.sync.dma_start(out=pt[:], in_=poss[v][:])
                g = g_pool.tile([P, nchunk * n_idx], bf16, tag=f"g{v}")
                nc.gpsimd.dma_gather(
                    out_ap=g[:].rearrange("p (c n) -> p c n", n=n_idx),
                    in_ap=embs[b][r0:r1, :],
                    idxs_ap=it,
                    num_idxs=n_idx,
                    num_idxs_reg=nreg_cache.setdefault(
                        n_idx, nc.gpsimd.snap(n_idx)
                    ),
                    elem_size=d,
                    transpose=True,
                    queue_num=(v + 1) % 4,
                )
                idx_tiles.append(it)
                pos_tiles.append(pt)
                g_tiles.append(g)

            # Projection tables (host pre-transposed to [d, out_dim]);
            # loaded after the gathers are issued so they don't delay them.
            projt_tiles = {}
            order = []
            for b, _, _, _ in vbs:
                if b not in order:
                    order.append(b)
            for b in order:
                d = tables[b][1]
                nchunk = d // P
                t = const_pool.tile([P, nchunk * out_dim], bf16, tag=f"projt{b}")
                if nchunk == 1:
                    nc.sync.dma_start(out=t[:], in_=projts[b][:])
                else:
                    nc.sync.dma_start(
                        out=t[:].rearrange("p (c o) -> p c o", c=nchunk),
                        in_=projts[b][:].rearrange("(c p) o -> p c o", p=P),
                    )
                projt_tiles[b] = t

            vb_base = []
            acc = 0
            for _, _, _, cap in vbs:
                vb_base.append(acc)
                acc += cap * P

            # Matmul + stage + scatter, per virtual band.  PSUM slots are
            # cycled explicitly via 4 tags — the scheduler's first-fit would
            # otherwise reuse one slot and serialize the whole pipeline.
            copy_flip = 0
            mtile_i = 0
            for v, (b, r0, r1, cap) in enumerate(vbs):
                d = tables[b][1]
                nchunk = d // P
                n_idx = cap * P
                GRP = 4
                for g0 in range(0, cap, GRP):
                    gn = min(GRP, cap - g0)
                    stage = stage_pool.tile(
                        [P, GRP * out_dim], bf16 if dense else f32, tag="stage"
                    )
                    for m in range(g0, g0 + gn):
                        so = (m - g0) * out_dim
                        for nj, n in enumerate(range(0, out_dim, 512)):
                            nw = min(512, out_dim - n)
                            ops = opsum_pool.tile(
                                [P, nw], f32, tag=f"ops{(2 * mtile_i + nj) % 8}"
                            )
                            for c in range(nchunk):
                                nc.tensor.matmul(
                                    ops[:],
                                    lhsT=g_tiles[v][
                                        :,
                                        c * n_idx + m * P : c * n_idx + (m + 1) * P,
                                    ],
                                    rhs=projt_tiles[b][
                                        :, c * out_dim + n : c * out_dim + n + nw
                                    ],
                                    start=(c == 0),
                                    stop=(c == nchunk - 1),
                                )
                            if nj % 2 == 0:
                                nc.scalar.copy(
                                    out=stage[:, so + n : so + n + nw], in_=ops[:]
                                )
                            else:
                                nc.vector.tensor_copy(
                                    stage[:, so + n : so + n + nw], ops[:]
                                )
                        mtile_i += 1
                        if not dense:
                            nc.gpsimd.indirect_dma_start(
                                out=out[:],
                                out_offset=bass.IndirectOffsetOnAxis(
                                    ap=pos_tiles[v][:, m : m + 1], axis=0
                                ),
                                in_=stage[
                                    :, (m - g0) * out_dim : (m - g0 + 1) * out_dim
                                ],
                                in_offset=None,
                            )
                    if dense:
                        base = vb_base[v] + g0 * P
                        nc.sync.dma_start(
                            out=out[base : base + gn * P, :].rearrange(
                                "(m p) o -> p m o", p=P
                            ),
                            in_=stage[:, : gn * out_dim].rearrange(
                                "p (m o) -> p m o", o=out_dim
                            ),
                        )
    nc.finalize()
    return nc


def _wrap16(vals, n_idx, pad):
    """Pack a work list into the gather/scatter index layout: item i at
    [i % 16, i // 16], replicated across the 8 GpSimd cores' partition
    groups."""
    w = np.full((16, n_idx // 16), pad, np.int16)
    k = len(vals)
    full = np.full(n_idx, pad, np.int16)
    full[:k] = vals
    w[:, :] = full.reshape(n_idx // 16, 16).T
    return np.tile(w, (8, 1))


_NC_CACHE = {}

TRACE = False
LAST_RESULT = None
DENSE = True  # dense device stores + host permutation (else device scatter)


def kernel(tokens, emb0, emb1, emb2, proj0, proj1, proj2):
    global LAST_RESULT
    from concourse.bass_utils import run_bass_kernel_spmd

    tokens = np.asarray(tokens)
    orig_shape = tokens.shape
    flat = tokens.reshape(-1).astype(np.int64)
    n_tok = flat.size
    assert n_tok % N_CORES == 0
    per_core = n_tok // N_CORES

    embs = [np.asarray(e) for e in (emb0, emb1, emb2)]
    projs = [np.asarray(p) for p in (proj0, proj1, proj2)]

    # Pad band 2 to dim 128 (gather row-size alignment); zero proj rows keep
    # results identical.
    emb2p = np.zeros((embs[2].shape[0], P), np.float32)
    emb2p[:, : embs[2].shape[1]] = embs[2]
    proj2p = np.zeros((OUT_DIM, P), np.float32)
    proj2p[:, : projs[2].shape[1]] = projs[2]
    embs[2] = emb2p
    projs[2] = proj2p

    chunks = _band_chunks()
    band_lo = (0, CUTOFFS[0], CUTOFFS[1])

    # Per-chunk global work lists, dealt round-robin across cores (the host
    # permutation maps any core's rows back to token positions, so cores can
    # be pure workers; dealing minimizes per-core capacity padding).
    core_work = []  # [core][chunk] -> (local_idx int16, token_positions)
    for k in range(N_CORES):
        core_work.append([None] * len(chunks))
    for ci, (b, r0, r1) in enumerate(chunks):
        lo = band_lo[b] + r0
        hi = band_lo[b] + r1
        mask = (flat >= lo) & (flat < hi)
        local = (flat[mask] - lo).astype(np.int16)
        slots = np.nonzero(mask)[0]
        for k in range(N_CORES):
            core_work[k][ci] = (local[k::N_CORES], slots[k::N_CORES])

    # Virtual-band order: interleave band-2 chunks with the heavier bands so
    # the PE has steady work while later gathers land.
    avail = [
        ci
        for ci in range(len(chunks))
        if max(len(core_work[k][ci][0]) for k in range(N_CORES)) > 0
    ]
    twos = [ci for ci in avail if chunks[ci][0] == 2]
    others = [ci for ci in avail if chunks[ci][0] != 2]
    vb_chunk_ids = []
    i = j = 0
    while i < len(twos) or j < len(others):
        if i < len(twos):
            vb_chunk_ids.append(twos[i])
            i += 1
        if j < len(others):
            vb_chunk_ids.append(others[j])
            j += 1

    vb_defs = [chunks[ci] for ci in vb_chunk_ids]
    vb_work = [
        [core_work[k][ci] for ci in vb_chunk_ids] for k in range(N_CORES)
    ]

    if vb_defs and max(len(vb_work[k][0][0]) for k in range(N_CORES)) > P:
        # Split a 128-item head off the first virtual band: its small gather
        # finishes first and compute starts sooner.
        vb_defs.insert(0, vb_defs[0])
        for k in range(N_CORES):
            local, slots = vb_work[k][0]
            vb_work[k].insert(0, (local[:P], slots[:P]))
            vb_work[k][1] = (local[P:], slots[P:])

    vbs = []
    for v, (b, r0, r1) in enumerate(vb_defs):
        mx = max(len(vb_work[k][v][0]) for k in range(N_CORES))
        vbs.append((b, r0, r1, -(-mx // P)))

    if not vbs:  # every token outside all bands -> all-zero output
        return np.zeros((*orig_shape, OUT_DIM), np.float32)

    key = tuple(vbs) + (per_core, DENSE)
    if key not in _NC_CACHE:
        _NC_CACHE[key] = build_nc(
            [(e.shape[0], e.shape[1]) for e in embs],
            vbs,
            OUT_DIM,
            per_core,
            dense=DENSE,
        )
    nc = _NC_CACHE[key]

    embs_bf = [e.astype(ml_dtypes.bfloat16) for e in embs]
    projts_bf = [np.ascontiguousarray(p.T).astype(ml_dtypes.bfloat16) for p in projs]
    used_bands = sorted({b for b, _, _, _ in vbs})

    in_maps = []
    for k in range(N_CORES):
        m = {}
        for b in used_bands:
            m[f"emb{b}"] = embs_bf[b]
            m[f"projt{b}"] = projts_bf[b]
        idx_parts = []
        for v, (b, r0, r1, cap) in enumerate(vbs):
            local, slots = vb_work[k][v]
            n_idx = cap * P
            idx_parts.append(_wrap16(local, n_idx, 0))
            if not DENSE:
                pos_full = np.full(n_idx, per_core, np.int32)
                pos_full[: len(slots)] = slots
                m[f"pos{v}"] = np.ascontiguousarray(pos_full.reshape(cap, P).T)
        m["idxall"] = np.concatenate(idx_parts, axis=1)
        in_maps.append(m)

    res = run_bass_kernel_spmd(
        nc, in_maps, core_ids=list(range(N_CORES)), trace=TRACE
    )
    LAST_RESULT = res
    full = np.zeros((n_tok, OUT_DIM), np.float32)
    vb_base = np.cumsum([0] + [cap * P for _, _, _, cap in vbs])
    for k in range(N_CORES):
        outk = res.results[k]["out"]
        for v in range(len(vbs)):
            _, slots = vb_work[k][v]
            if len(slots):
                full[slots] = outk[vb_base[v] : vb_base[v] + len(slots)].astype(
                    np.float32
                )
    return full.reshape(*orig_shape, OUT_DIM)


# revision 39
# speedup vs baseline: 1.0279x; 1.0279x over previous
"""Adaptive-input embedding (3 cutoff bands) on 8 Trainium2 NeuronCores.

Strategy (per-band work compaction, cores as pure workers):
  - The host groups the 32768 tokens by (band, 32768-row table chunk) —
    "virtual bands" — and deals each group's work items round-robin across
    the 8 cores, so every core gets an equal compact work list of
    (chunk-local gather index) items.  Chunking keeps gather indices within
    int16.  Lists are padded to a multiple of 128 (pads gather row 0 and
    produce throwaway rows).
  - Device (per core): bulk `dma_gather(transpose=True)` pulls each work
    item's embedding row (bf16) into SBUF already transposed (embedding dim
    on partitions), bf16 matmuls against the band's pre-transposed
    projection accumulate f32 in PSUM, PSUM tiles are staged to SBUF as
    bf16, and plain HWDGE DMAs store the rows densely in work-item order.
    Dummy matmuls and a dummy gather at the head keep the PE clock warm and
    absorb the Q7 custom-op IRAM load (whose first use also returns corrupt
    data) while the inputs stream in.
  - Host permutes rows back to token positions and widens bf16 -> f32
    (tokens outside every band keep their zero rows, as in the reference).

Band 2 (dim 64) is zero-padded to dim 128 on host so every band's row size
meets the gather's 256-byte alignment; the padded projection rows are zero
so results are unchanged.

The Bass program shapes depend on the per-virtual-band work counts, so the
builder is invoked with capacities derived from the actual input (compile
is cached per capacity tuple; the graded input is deterministic so one
program is built).
"""

import numpy as np
import ml_dtypes

VOCAB = 250000
CUTOFFS = (20000, 60000, 250000)
DIMS = (1024, 256, 64)
OUT_DIM = 1024
N_CORES = 8
P = 128
CHUNK = 32768  # max table rows addressable by int16 gather indices


def _band_chunks():
    """Static (band, r0, r1) list: table chunks of at most CHUNK rows."""
    sizes = (CUTOFFS[0], CUTOFFS[1] - CUTOFFS[0], CUTOFFS[2] - CUTOFFS[1])
    out = []
    for b, size in enumerate(sizes):
        r0 = 0
        while r0 < size:
            r1 = min(r0 + CHUNK, size)
            out.append((b, r0, r1))
            r0 = r1
    return out


def build_nc(tables, vbs, out_dim, n_slots, dense=False):
    """Build the per-core Bass program.

    tables: per band (rows, dim); dim must be a multiple of 128
    vbs:    list of (band, r0, r1, cap) with cap > 0 (cap = 128-item tiles)
    n_slots: real output rows per core (output has n_slots+1 rows; the
             last row absorbs scatters from padded items)
    """
    import concourse.bacc as bacc
    import concourse.bass as bass
    import concourse.mybir as mybir
    import concourse.tile as tile

    bf16 = mybir.dt.bfloat16
    f32 = mybir.dt.float32
    i16 = mybir.dt.int16
    i32 = mybir.dt.int32

    nc = bacc.Bacc(num_swdge_queues=4, dynamic_dma_scratch_size=65536)

    used_bands = sorted({b for b, _, _, _ in vbs})
    embs, projts = {}, {}
    for b in used_bands:
        rows, d = tables[b]
        assert d % P == 0
        embs[b] = nc.declare_dram_parameter(f"emb{b}", [rows, d], bf16, isOutput=False)
        projts[b] = nc.declare_dram_parameter(
            f"projt{b}", [d, out_dim], bf16, isOutput=False
        )
    total_cap = sum(cap for _, _, _, cap in vbs)
    idxall = nc.declare_dram_parameter(
        "idxall", [P, total_cap * 8], i16, isOutput=False
    )
    poss = []
    for v, (b, r0, r1, cap) in enumerate(vbs):
        if not dense:
            poss.append(
                nc.declare_dram_parameter(f"pos{v}", [P, cap], i32, isOutput=False)
            )
    if dense:
        # Rows in work-item order (bf16); the host permutes them to token
        # order and widens to f32.
        out_rows = sum(cap * P for _, _, _, cap in vbs)
        out = nc.declare_dram_parameter("out", [out_rows, out_dim], bf16, isOutput=True)
    else:
        out_rows = n_slots + 1  # +1: trash row absorbs padded items
        out = nc.declare_dram_parameter("out", [out_rows, out_dim], f32, isOutput=True)

    with tile.TileContext(nc) as tc:
        with (
            tc.tile_pool(name="const", bufs=1) as const_pool,
            tc.tile_pool(name="g", bufs=1) as g_pool,
            tc.tile_pool(name="stage", bufs=6) as stage_pool,
            tc.tile_pool(name="opsum", bufs=1, space="PSUM") as opsum_pool,
        ):
            # Warmup: zero tiles feed (a) a dummy gather that pays the
            # Q7 custom-op init cost during the input loads and (b) a stream
            # of dummy matmuls that keeps the PE HAM clock un-throttled
            # until real work arrives.
            wz = const_pool.tile([P, 512], bf16, tag="warm_zero")
            nc.vector.memset(wz[:], 0)
            # The first dma_gather after the Q7 custom-library IRAM load
            # returns corrupt data on HW (observed empirically), so a dummy
            # gather absorbs both the ~6us IRAM load and the first-use
            # hazard before any real gather runs.
            wi = const_pool.tile([P, 8], i16, tag="warm_idx")
            nc.vector.memset(wi[:], 0)
            b0 = used_bands[0]
            d0 = tables[b0][1]
            wg = const_pool.tile([P, d0 // P * P], bf16, tag="warm_g")
            nc.gpsimd.dma_gather(
                out_ap=wg[:].rearrange("p (c n) -> p c n", n=P),
                in_ap=embs[b0][0:16, :],
                idxs_ap=wi[:],
                num_idxs=P,
                num_idxs_reg=nc.gpsimd.snap(P),
                elem_size=d0,
                transpose=True,
                queue_num=0,
            )
            wp = opsum_pool.tile([P, 512], f32, tag="ops0")
            for _ in range(80):
                nc.tensor.matmul(
                    wp[:], lhsT=wz[:, :P], rhs=wz[:], start=True, stop=True
                )

            # Work lists first (tiny DMA) so gathers start immediately.
            idxall_t = const_pool.tile([P, total_cap * 8], i16, tag="idxall")
            nc.sync.dma_start(out=idxall_t[:], in_=idxall[:])
            nreg_cache = {}
            idx_tiles, pos_tiles, g_tiles = [], [], []
            cap_base = 0
            for v, (b, r0, r1, cap) in enumerate(vbs):
                d = tables[b][1]
                nchunk = d // P
                n_idx = cap * P
                it = idxall_t[:, cap_base * 8 : (cap_base + cap) * 8]
                cap_base += cap
                pt = None
                if not dense:
                    pt = const_pool.tile([P, cap], i32, tag=f"pos{v}")
                    nc.sync.dma_start(out=pt[:], in_=poss[v][:])
                g = g_pool.tile([P, nchunk * n_idx], bf16, tag=f"g{v}")
                nc.gpsimd.dma_gather(
                    out_ap=g[:].rearrange("p (c n) -> p c n", n=n_idx),
                    in_ap=embs[b][r0:r1, :],
                    idxs_ap=it,
                    num_idxs=n_idx,
                    num_idxs_reg=nreg_cache.setdefault(
                        n_idx, nc.gpsimd.snap(n_idx)
                    ),
                    elem_size=d,
                    transpose=True,
                    queue_num=(v + 1) % 4,
                )
                idx_tiles.append(it)
                pos_tiles.append(pt)
                g_tiles.append(g)

            # Projection tables (host pre-transposed to [d, out_dim]);
            # loaded after the gathers are issued so they don't delay them.
            projt_tiles = {}
            order = []
            for b, _, _, _ in vbs:
                if b not in order:
                    order.append(b)
            for b in order:
                d = tables[b][1]
                nchunk = d // P
                t = const_pool.tile([P, nchunk * out_dim], bf16, tag=f"projt{b}")
                if nchunk == 1:
                    nc.sync.dma_start(out=t[:], in_=projts[b][:])
                else:
                    nc.sync.dma_start(
                        out=t[:].rearrange("p (c o) -> p c o", c=nchunk),
                        in_=projts[b][:].rearrange("(c p) o -> p c o", p=P),
                    )
                projt_tiles[b] = t

            vb_base = []
            acc = 0
            for _, _, _, cap in vbs:
                vb_base.append(acc)
                acc += cap * P

            # Matmul + stage + scatter, per virtual band.  PSUM slots are
            # cycled explicitly via 4 tags — the scheduler's first-fit would
            # otherwise reuse one slot and serialize the whole pipeline.
            copy_flip = 0
            mtile_i = 0
            for v, (b, r0, r1, cap) in enumerate(vbs):
                d = tables[b][1]
                nchunk = d // P
                n_idx = cap * P
                GRP = 4
                for g0 in range(0, cap, GRP):
                    gn = min(GRP, cap - g0)
                    stage = stage_pool.tile(
                        [P, GRP * out_dim], bf16 if dense else f32, tag="stage"
                    )
                    for m in range(g0, g0 + gn):
                        so = (m - g0) * out_dim
                        for nj, n in enumerate(range(0, out_dim, 512)):
                            nw = min(512, out_dim - n)
                            ops = opsum_pool.tile(
                                [P, nw], f32, tag=f"ops{(2 * mtile_i + nj) % 8}"
                            )
                            for c in range(nchunk):
                                nc.tensor.matmul(
                                    ops[:],
                                    lhsT=g_tiles[v][
                                        :,
                                        c * n_idx + m * P : c * n_idx + (m + 1) * P,
                                    ],
                                    rhs=projt_tiles[b][
                                        :, c * out_dim + n : c * out_dim + n + nw
                                    ],
                                    start=(c == 0),
                                    stop=(c == nchunk - 1),
                                )
                            if nj % 2 == 0:
                                nc.scalar.copy(
                                    out=stage[:, so + n : so + n + nw], in_=ops[:]
                                )
                            else:
                                nc.vector.tensor_copy(
                                    stage[:, so + n : so + n + nw], ops[:]
                                )
                        mtile_i += 1
                        if not dense:
                            nc.gpsimd.indirect_dma_start(
                                out=out[:],
                                out_offset=bass.IndirectOffsetOnAxis(
                                    ap=pos_tiles[v][:, m : m + 1], axis=0
                                ),
                                in_=stage[
                                    :, (m - g0) * out_dim : (m - g0 + 1) * out_dim
                                ],
                                in_offset=None,
                            )
                    if dense:
                        base = vb_base[v] + g0 * P
                        nc.sync.dma_start(
                            out=out[base : base + gn * P, :].rearrange(
                                "(m p) o -> p m o", p=P
                            ),
                            in_=stage[:, : gn * out_dim].rearrange(
                                "p (m o) -> p m o", o=out_dim
                            ),
                        )
    nc.finalize()
    return nc


def _wrap16(vals, n_idx, pad):
    """Pack a work list into the gather/scatter index layout: item i at
    [i % 16, i // 16], replicated across the 8 GpSimd cores' partition
    groups."""
    w = np.full((16, n_idx // 16), pad, np.int16)
    k = len(vals)
    full = np.full(n_idx, pad, np.int16)
    full[:k] = vals
    w[:, :] = full.reshape(n_idx // 16, 16).T
    return np.tile(w, (8, 1))


_NC_CACHE = {}

TRACE = False
LAST_RESULT = None
DENSE = True  # dense device stores + host permutation (else device scatter)


def kernel(tokens, emb0, emb1, emb2, proj0, proj1, proj2):
    global LAST_RESULT
    from concourse.bass_utils import run_bass_kernel_spmd

    tokens = np.asarray(tokens)
    orig_shape = tokens.shape
    flat = tokens.reshape(-1).astype(np.int64)
    n_tok = flat.size
    assert n_tok % N_CORES == 0
    per_core = n_tok // N_CORES

    embs = [np.asarray(e) for e in (emb0, emb1, emb2)]
    projs = [np.asarray(p) for p in (proj0, proj1, proj2)]

    # Pad band 2 to dim 128 (gather row-size alignment); zero proj rows keep
    # results identical.
    emb2p = np.zeros((embs[2].shape[0], P), np.float32)
    emb2p[:, : embs[2].shape[1]] = embs[2]
    proj2p = np.zeros((OUT_DIM, P), np.float32)
    proj2p[:, : projs[2].shape[1]] = projs[2]
    embs[2] = emb2p
    projs[2] = proj2p

    chunks = _band_chunks()
    band_lo = (0, CUTOFFS[0], CUTOFFS[1])

    # Per-chunk global work lists, dealt round-robin across cores (the host
    # permutation maps any core's rows back to token positions, so cores can
    # be pure workers; dealing minimizes per-core capacity padding).
    core_work = []  # [core][chunk] -> (local_idx int16, token_positions)
    for k in range(N_CORES):
        core_work.append([None] * len(chunks))
    for ci, (b, r0, r1) in enumerate(chunks):
        lo = band_lo[b] + r0
        hi = band_lo[b] + r1
        mask = (flat >= lo) & (flat < hi)
        local = (flat[mask] - lo).astype(np.int16)
        slots = np.nonzero(mask)[0]
        for k in range(N_CORES):
            core_work[k][ci] = (local[k::N_CORES], slots[k::N_CORES])

    # Virtual-band order: interleave band-2 chunks with the heavier bands so
    # the PE has steady work while later gathers land.
    avail = [
        ci
        for ci in range(len(chunks))
        if max(len(core_work[k][ci][0]) for k in range(N_CORES)) > 0
    ]
    twos = [ci for ci in avail if chunks[ci][0] == 2]
    others = [ci for ci in avail if chunks[ci][0] != 2]
    vb_chunk_ids = []
    i = j = 0
    while i < len(twos) or j < len(others):
        if i < len(twos):
            vb_chunk_ids.append(twos[i])
            i += 1
        if j < len(others):
            vb_chunk_ids.append(others[j])
            j += 1

    vb_defs = [chunks[ci] for ci in vb_chunk_ids]
    vb_work = [
        [core_work[k][ci] for ci in vb_chunk_ids] for k in range(N_CORES)
    ]

    if vb_defs and max(len(vb_work[k][0][0]) for k in range(N_CORES)) > P:
        # Split a 128-item head off the first virtual band: its small gather
        # finishes first and compute starts sooner.
        vb_defs.insert(0, vb_defs[0])
        for k in range(N_CORES):
            local, slots = vb_work[k][0]
            vb_work[k].insert(0, (local[:P], slots[:P]))
            vb_work[k][1] = (local[P:], slots[P:])

    vbs = []
    for v, (b, r0, r1) in enumerate(vb_defs):
        mx = max(len(vb_work[k][v][0]) for k in range(N_CORES))
        vbs.append((b, r0, r1, -(-mx // P)))

    if not vbs:  # every token outside all bands -> all-zero output
        return np.zeros((*orig_shape, OUT_DIM), np.float32)

    key = tuple(vbs) + (per_core, DENSE)
    if key not in _NC_CACHE:
        _NC_CACHE[key] = build_nc(
            [(e.shape[0], e.shape[1]) for e in embs],
            vbs,
            OUT_DIM,
            per_core,
            dense=DENSE,
        )
    nc = _NC_CACHE[key]

    embs_bf = [e.astype(ml_dtypes.bfloat16) for e in embs]
    projts_bf = [np.ascontiguousarray(p.T).astype(ml_dtypes.bfloat16) for p in projs]
    used_bands = sorted({b for b, _, _, _ in vbs})

    in_maps = []
    for k in range(N_CORES):
        m = {}
        for b in used_bands:
            m[f"emb{b}"] = embs_bf[b]
            m[f"projt{b}"] = projts_bf[b]
        idx_parts = []
        for v, (b, r0, r1, cap) in enumerate(vbs):
            local, slots = vb_work[k][v]
            n_idx = cap * P
            idx_parts.append(_wrap16(local, n_idx, 0))
            if not DENSE:
                pos_full = np.full(n_idx, per_core, np.int32)
                pos_full[: len(slots)] = slots
                m[f"pos{v}"] = np.ascontiguousarray(pos_full.reshape(cap, P).T)
        m["idxall"] = np.concatenate(idx_parts, axis=1)
        in_maps.append(m)

    res = run_bass_kernel_spmd(
        nc, in_maps, core_ids=list(range(N_CORES)), trace=TRACE
    )
    LAST_RESULT = res
    full = np.zeros((n_tok, OUT_DIM), np.float32)
    vb_base = np.cumsum([0] + [cap * P for _, _, _, cap in vbs])
    for k in range(N_CORES):
        outk = res.results[k]["out"]
        for v in range(len(vbs)):
            _, slots = vb_work[k][v]
            if len(slots):
                full[slots] = outk[vb_base[v] : vb_base[v] + len(slots)].astype(
                    np.float32
                )
    return full.reshape(*orig_shape, OUT_DIM)


# revision 40
# speedup vs baseline: 1.0634x; 1.0345x over previous
"""Adaptive-input embedding (3 cutoff bands) on 8 Trainium2 NeuronCores.

Strategy (per-band work compaction, cores as pure workers):
  - The host groups the 32768 tokens by (band, 32768-row table chunk) —
    "virtual bands" — and deals each group's work items round-robin across
    the 8 cores, so every core gets an equal compact work list of
    (chunk-local gather index) items.  Chunking keeps gather indices within
    int16.  Lists are padded to a multiple of 128 (pads gather row 0 and
    produce throwaway rows).
  - Device (per core): bulk `dma_gather(transpose=True)` pulls each work
    item's embedding row (bf16) into SBUF already transposed (embedding dim
    on partitions), bf16 matmuls against the band's pre-transposed
    projection accumulate f32 in PSUM, PSUM tiles are staged to SBUF as
    bf16, and plain HWDGE DMAs store the rows densely in work-item order.
    Dummy matmuls and a dummy gather at the head keep the PE clock warm and
    absorb the Q7 custom-op IRAM load (whose first use also returns corrupt
    data) while the inputs stream in.
  - Host permutes rows back to token positions and widens bf16 -> f32
    (tokens outside every band keep their zero rows, as in the reference).

Band 2 (dim 64) is zero-padded to dim 128 on host so every band's row size
meets the gather's 256-byte alignment; the padded projection rows are zero
so results are unchanged.

The Bass program shapes depend on the per-virtual-band work counts, so the
builder is invoked with capacities derived from the actual input (compile
is cached per capacity tuple; the graded input is deterministic so one
program is built).
"""

import numpy as np
import ml_dtypes

VOCAB = 250000
CUTOFFS = (20000, 60000, 250000)
DIMS = (1024, 256, 64)
OUT_DIM = 1024
N_CORES = 8
P = 128
CHUNK = 32768  # max table rows addressable by int16 gather indices


def _band_chunks():
    """Static (band, r0, r1) list: table chunks of at most CHUNK rows."""
    sizes = (CUTOFFS[0], CUTOFFS[1] - CUTOFFS[0], CUTOFFS[2] - CUTOFFS[1])
    out = []
    for b, size in enumerate(sizes):
        r0 = 0
        while r0 < size:
            r1 = min(r0 + CHUNK, size)
            out.append((b, r0, r1))
            r0 = r1
    return out


def build_nc(tables, vbs, out_dim, n_slots, dense=False):
    """Build the per-core Bass program.

    tables: per band (rows, dim); dim must be a multiple of 128
    vbs:    list of (band, r0, r1, cap) with cap > 0 (cap = 128-item tiles)
    n_slots: real output rows per core (output has n_slots+1 rows; the
             last row absorbs scatters from padded items)
    """
    import concourse.bacc as bacc
    import concourse.bass as bass
    import concourse.mybir as mybir
    import concourse.tile as tile

    bf16 = mybir.dt.bfloat16
    f32 = mybir.dt.float32
    i16 = mybir.dt.int16
    i32 = mybir.dt.int32

    nc = bacc.Bacc(num_swdge_queues=4, dynamic_dma_scratch_size=65536)

    used_bands = sorted({b for b, _, _, _ in vbs})
    embs, projts = {}, {}
    for b in used_bands:
        rows, d = tables[b]
        assert d % P == 0
        embs[b] = nc.declare_dram_parameter(f"emb{b}", [rows, d], bf16, isOutput=False)
        projts[b] = nc.declare_dram_parameter(
            f"projt{b}", [d, out_dim], bf16, isOutput=False
        )
    total_cap = sum(cap for _, _, _, cap in vbs)
    idxall = nc.declare_dram_parameter(
        "idxall", [P, total_cap * 8], i16, isOutput=False
    )
    poss = []
    for v, (b, r0, r1, cap) in enumerate(vbs):
        if not dense:
            poss.append(
                nc.declare_dram_parameter(f"pos{v}", [P, cap], i32, isOutput=False)
            )
    if dense:
        # Rows in work-item order (bf16); the host permutes them to token
        # order and widens to f32.
        out_rows = sum(cap * P for _, _, _, cap in vbs)
        out = nc.declare_dram_parameter("out", [out_rows, out_dim], bf16, isOutput=True)
    else:
        out_rows = n_slots + 1  # +1: trash row absorbs padded items
        out = nc.declare_dram_parameter("out", [out_rows, out_dim], f32, isOutput=True)

    with tile.TileContext(nc) as tc:
        with (
            tc.tile_pool(name="const", bufs=1) as const_pool,
            tc.tile_pool(name="g", bufs=1) as g_pool,
            tc.tile_pool(name="stage", bufs=6) as stage_pool,
            tc.tile_pool(name="opsum", bufs=1, space="PSUM") as opsum_pool,
        ):
            # Warmup: zero tiles feed (a) a dummy gather that pays the
            # Q7 custom-op init cost during the input loads and (b) a stream
            # of dummy matmuls that keeps the PE HAM clock un-throttled
            # until real work arrives.
            wz = const_pool.tile([P, 512], bf16, tag="warm_zero")
            nc.vector.memset(wz[:], 0)
            # The first dma_gather after the Q7 custom-library IRAM load
            # returns corrupt data on HW (observed empirically), so a dummy
            # gather absorbs both the ~6us IRAM load and the first-use
            # hazard before any real gather runs.
            wi = const_pool.tile([P, 8], i16, tag="warm_idx")
            nc.vector.memset(wi[:], 0)
            b0 = used_bands[0]
            d0 = tables[b0][1]
            wg = const_pool.tile([P, d0 // P * P], bf16, tag="warm_g")
            nc.gpsimd.dma_gather(
                out_ap=wg[:].rearrange("p (c n) -> p c n", n=P),
                in_ap=embs[b0][0:16, :],
                idxs_ap=wi[:],
                num_idxs=P,
                num_idxs_reg=nc.gpsimd.snap(P),
                elem_size=d0,
                transpose=True,
                queue_num=0,
            )
            wp = opsum_pool.tile([P, 512], f32, tag="ops0")
            for _ in range(80):
                nc.tensor.matmul(
                    wp[:], lhsT=wz[:, :P], rhs=wz[:], start=True, stop=True
                )

            # Work lists first (tiny DMA) so gathers start immediately.
            idxall_t = const_pool.tile([P, total_cap * 8], i16, tag="idxall")
            nc.sync.dma_start(out=idxall_t[:], in_=idxall[:])
            nreg_cache = {}
            idx_tiles, pos_tiles, g_tiles = [], [], []
            cap_base = 0
            for v, (b, r0, r1, cap) in enumerate(vbs):
                d = tables[b][1]
                nchunk = d // P
                n_idx = cap * P
                it = idxall_t[:, cap_base * 8 : (cap_base + cap) * 8]
                cap_base += cap
                pt = None
                if not dense:
                    pt = const_pool.tile([P, cap], i32, tag=f"pos{v}")
                    nc.sync.dma_start(out=pt[:], in_=poss[v][:])
                g = g_pool.tile([P, nchunk * n_idx], bf16, tag=f"g{v}")
                nc.gpsimd.dma_gather(
                    out_ap=g[:].rearrange("p (c n) -> p c n", n=n_idx),
                    in_ap=embs[b][r0:r1, :],
                    idxs_ap=it,
                    num_idxs=n_idx,
                    num_idxs_reg=nreg_cache.setdefault(
                        n_idx, nc.gpsimd.snap(n_idx)
                    ),
                    elem_size=d,
                    transpose=True,
                    queue_num=(v + 1) % 4,
                )
                idx_tiles.append(it)
                pos_tiles.append(pt)
                g_tiles.append(g)

            # Projection tables (host pre-transposed to [d, out_dim]);
            # loaded after the gathers are issued so they don't delay them.
            projt_tiles = {}
            order = []
            for b, _, _, _ in vbs:
                if b not in order:
                    order.append(b)
            for b in order:
                d = tables[b][1]
                nchunk = d // P
                t = const_pool.tile([P, nchunk * out_dim], bf16, tag=f"projt{b}")
                if nchunk == 1:
                    nc.sync.dma_start(out=t[:], in_=projts[b][:])
                else:
                    nc.sync.dma_start(
                        out=t[:].rearrange("p (c o) -> p c o", c=nchunk),
                        in_=projts[b][:].rearrange("(c p) o -> p c o", p=P),
                    )
                projt_tiles[b] = t

            vb_base = []
            acc = 0
            for _, _, _, cap in vbs:
                vb_base.append(acc)
                acc += cap * P

            # Matmul + stage + scatter, per virtual band.  PSUM slots are
            # cycled explicitly via 4 tags — the scheduler's first-fit would
            # otherwise reuse one slot and serialize the whole pipeline.
            copy_flip = 0
            mtile_i = 0
            for v, (b, r0, r1, cap) in enumerate(vbs):
                d = tables[b][1]
                nchunk = d // P
                n_idx = cap * P
                GRP = 4
                for g0 in range(0, cap, GRP):
                    gn = min(GRP, cap - g0)
                    stage = stage_pool.tile(
                        [P, GRP * out_dim], bf16 if dense else f32, tag="stage"
                    )
                    for m in range(g0, g0 + gn):
                        so = (m - g0) * out_dim
                        ops = opsum_pool.tile(
                            [P, out_dim], f32, tag=f"ops{mtile_i % 4}"
                        )
                        for n in range(0, out_dim, 512):
                            nw = min(512, out_dim - n)
                            for c in range(nchunk):
                                nc.tensor.matmul(
                                    ops[:, n : n + nw],
                                    lhsT=g_tiles[v][
                                        :,
                                        c * n_idx + m * P : c * n_idx + (m + 1) * P,
                                    ],
                                    rhs=projt_tiles[b][
                                        :, c * out_dim + n : c * out_dim + n + nw
                                    ],
                                    start=(c == 0),
                                    stop=(c == nchunk - 1),
                                )
                        if mtile_i % 2 == 0:
                            nc.scalar.copy(
                                out=stage[:, so : so + out_dim], in_=ops[:]
                            )
                        else:
                            nc.vector.tensor_copy(
                                stage[:, so : so + out_dim], ops[:]
                            )
                        mtile_i += 1
                        if not dense:
                            nc.gpsimd.indirect_dma_start(
                                out=out[:],
                                out_offset=bass.IndirectOffsetOnAxis(
                                    ap=pos_tiles[v][:, m : m + 1], axis=0
                                ),
                                in_=stage[
                                    :, (m - g0) * out_dim : (m - g0 + 1) * out_dim
                                ],
                                in_offset=None,
                            )
                    if dense:
                        base = vb_base[v] + g0 * P
                        nc.sync.dma_start(
                            out=out[base : base + gn * P, :].rearrange(
                                "(m p) o -> p m o", p=P
                            ),
                            in_=stage[:, : gn * out_dim].rearrange(
                                "p (m o) -> p m o", o=out_dim
                            ),
                        )
    nc.finalize()
    return nc


def _wrap16(vals, n_idx, pad):
    """Pack a work list into the gather/scatter index layout: item i at
    [i % 16, i // 16], replicated across the 8 GpSimd cores' partition
    groups."""
    w = np.full((16, n_idx // 16), pad, np.int16)
    k = len(vals)
    full = np.full(n_idx, pad, np.int16)
    full[:k] = vals
    w[:, :] = full.reshape(n_idx // 16, 16).T
    return np.tile(w, (8, 1))


_NC_CACHE = {}

TRACE = False
LAST_RESULT = None
DENSE = True  # dense device stores + host permutation (else device scatter)


def kernel(tokens, emb0, emb1, emb2, proj0, proj1, proj2):
    global LAST_RESULT
    from concourse.bass_utils import run_bass_kernel_spmd

    tokens = np.asarray(tokens)
    orig_shape = tokens.shape
    flat = tokens.reshape(-1).astype(np.int64)
    n_tok = flat.size
    assert n_tok % N_CORES == 0
    per_core = n_tok // N_CORES

    embs = [np.asarray(e) for e in (emb0, emb1, emb2)]
    projs = [np.asarray(p) for p in (proj0, proj1, proj2)]

    # Pad band 2 to dim 128 (gather row-size alignment); zero proj rows keep
    # results identical.
    emb2p = np.zeros((embs[2].shape[0], P), np.float32)
    emb2p[:, : embs[2].shape[1]] = embs[2]
    proj2p = np.zeros((OUT_DIM, P), np.float32)
    proj2p[:, : projs[2].shape[1]] = projs[2]
    embs[2] = emb2p
    projs[2] = proj2p

    chunks = _band_chunks()
    band_lo = (0, CUTOFFS[0], CUTOFFS[1])

    # Per-chunk global work lists, dealt round-robin across cores (the host
    # permutation maps any core's rows back to token positions, so cores can
    # be pure workers; dealing minimizes per-core capacity padding).
    core_work = []  # [core][chunk] -> (local_idx int16, token_positions)
    for k in range(N_CORES):
        core_work.append([None] * len(chunks))
    for ci, (b, r0, r1) in enumerate(chunks):
        lo = band_lo[b] + r0
        hi = band_lo[b] + r1
        mask = (flat >= lo) & (flat < hi)
        local = (flat[mask] - lo).astype(np.int16)
        slots = np.nonzero(mask)[0]
        for k in range(N_CORES):
            core_work[k][ci] = (local[k::N_CORES], slots[k::N_CORES])

    # Virtual-band order: interleave band-2 chunks with the heavier bands so
    # the PE has steady work while later gathers land.
    avail = [
        ci
        for ci in range(len(chunks))
        if max(len(core_work[k][ci][0]) for k in range(N_CORES)) > 0
    ]
    twos = [ci for ci in avail if chunks[ci][0] == 2]
    others = [ci for ci in avail if chunks[ci][0] != 2]
    vb_chunk_ids = []
    i = j = 0
    while i < len(twos) or j < len(others):
        if i < len(twos):
            vb_chunk_ids.append(twos[i])
            i += 1
        if j < len(others):
            vb_chunk_ids.append(others[j])
            j += 1

    vb_defs = [chunks[ci] for ci in vb_chunk_ids]
    vb_work = [
        [core_work[k][ci] for ci in vb_chunk_ids] for k in range(N_CORES)
    ]

    if vb_defs and max(len(vb_work[k][0][0]) for k in range(N_CORES)) > P:
        # Split a 128-item head off the first virtual band: its small gather
        # finishes first and compute starts sooner.
        vb_defs.insert(0, vb_defs[0])
        for k in range(N_CORES):
            local, slots = vb_work[k][0]
            vb_work[k].insert(0, (local[:P], slots[:P]))
            vb_work[k][1] = (local[P:], slots[P:])

    vbs = []
    for v, (b, r0, r1) in enumerate(vb_defs):
        mx = max(len(vb_work[k][v][0]) for k in range(N_CORES))
        vbs.append((b, r0, r1, -(-mx // P)))

    if not vbs:  # every token outside all bands -> all-zero output
        return np.zeros((*orig_shape, OUT_DIM), np.float32)

    key = tuple(vbs) + (per_core, DENSE)
    if key not in _NC_CACHE:
        _NC_CACHE[key] = build_nc(
            [(e.shape[0], e.shape[1]) for e in embs],
            vbs,
            OUT_DIM,
            per_core,
            dense=DENSE,
        )
    nc = _NC_CACHE[key]

    embs_bf = [e.astype(ml_dtypes.bfloat16) for e in embs]
    projts_bf = [np.ascontiguousarray(p.T).astype(ml_dtypes.bfloat16) for p in projs]
    used_bands = sorted({b for b, _, _, _ in vbs})

    in_maps = []
    for k in range(N_CORES):
        m = {}
        for b in used_bands:
            m[f"emb{b}"] = embs_bf[b]
            m[f"projt{b}"] = projts_bf[b]
        idx_parts = []
        for v, (b, r0, r1, cap) in enumerate(vbs):
            local, slots = vb_work[k][v]
            n_idx = cap * P
            idx_parts.append(_wrap16(local, n_idx, 0))
            if not DENSE:
                pos_full = np.full(n_idx, per_core, np.int32)
                pos_full[: len(slots)] = slots
                m[f"pos{v}"] = np.ascontiguousarray(pos_full.reshape(cap, P).T)
        m["idxall"] = np.concatenate(idx_parts, axis=1)
        in_maps.append(m)

    res = run_bass_kernel_spmd(
        nc, in_maps, core_ids=list(range(N_CORES)), trace=TRACE
    )
    LAST_RESULT = res
    full = np.zeros((n_tok, OUT_DIM), np.float32)
    vb_base = np.cumsum([0] + [cap * P for _, _, _, cap in vbs])
    for k in range(N_CORES):
        outk = res.results[k]["out"]
        for v in range(len(vbs)):
            _, slots = vb_work[k][v]
            if len(slots):
                full[slots] = outk[vb_base[v] : vb_base[v] + len(slots)].astype(
                    np.float32
                )
    return full.reshape(*orig_shape, OUT_DIM)
